# revision 10
# baseline (speedup 1.0000x reference)
"""Trainium2 Bass kernel for nn_CrossAttention (B=8, N=M=2048, C=512, H=4).

Sharding: data-parallel over batch - one batch element per NeuronCore (8 cores).

v4. Two limits shape the schedule: the scalar engine's exp over all
N*M*H scores (16.8M elems ~ 137us at 128 lanes * 1.2 GHz) and the tensor
engine's matmul stream + unhidden per-matmul LDWEIGHTS cost. Both engines
are kept near-100% busy:

  1. Inputs land via two parallel DMA paths: F2 and F1-g1..3 as
     fp32->fp16 casting DMAs on the gpsimd queue; W / F1-g0 / W_proj as
     fp32 on the sync queue (fp32 PE transposes for F1-g0). kvT chunks
     interleave with F2^T per column group to fill the DMA wait.
  2. The qkv projection evacuates three ways from PSUM: qT8 (fp8e4,
     zero-padded by one stripe), kvT8z (fp8e4, zeroed second k-tile) -
     the DoubleRow scores operands - and kvT16 (fp16, transpose source
     for kvn2 = m-major kv mb-pairs [P, jj, 2, C] fp8).
  3. Attention per (stripe of 512, head), pipelined one iteration deep:
       scores: fp8 DoubleRow matmuls at 0.5 cyc/col - half the fp16
               cost; the second k-tile is annihilated by the zero half
               of kvT8z (junk second half of the moving operand is
               finite, so 0*x contributes exactly 0).
       exp:    ACT, PSUM->SBUF fp8e4 E (|SCALE*s| <= ~2, no max-sub).
       pv/dn:  fp8 DoubleRow over mb-pairs; dn via [128,2,32] ones into
               a [32,SW] PSUM row block.
     pv/dn of iteration t-1 interleave between the scores of iteration
     t (PSUM: sc 2x2 + pv 1 + dn 1 + pj 2 = 8 banks). The epilogue
     copies pv to SBUF first (frees its PSUM bank without waiting on
     the reciprocal chain), then reciprocal_approx_fast on [1,SW],
     GPSIMD partition-broadcast, DVE multiply into xT.
     Remaining phase-2 work (kvT heads 1-3, kvn, F1^T g1-3, qT chunks)
     and phase-4 output chunks drain through a fixed per-iteration
     filler schedule, filling PE slack under ACT's exp.
  4. out[n,c] = xT.T @ W_proj + b (fp16 PE; bias added by DVE from a
     pre-broadcast tile), spread one n-block at a time, DMA out.
"""
import sys

for _p in ("/opt/trn_rl_repo", "/root/.axon_site/_ro/trn_rl_repo"):
    if _p not in sys.path:
        sys.path.insert(0, _p)

import numpy as np
import concourse.bass as bass
import concourse.bacc as bacc
import concourse.tile as tile
from concourse import mybir
from concourse.bass_utils import run_bass_kernel_spmd

F32 = mybir.dt.float32
F16 = mybir.dt.float16
FP8 = mybir.dt.float8e4
EXP = mybir.ActivationFunctionType.Exp
DR = mybir.MatmulPerfMode.DoubleRow

B, N, M, C = 8, 2048, 2048, 512
H, D = 4, 128
SCALE = 1.0 / np.sqrt(C)
P = 128
NB = N // P        # 16 n-blocks
MB = M // P        # 16 m-blocks
KC = C // P        # 4 contraction chunks (= heads since D=128)
NS = 4             # n-stripes of 512
SW = N // NS       # stripe width 512
JJ = MB // 2       # 8 mb-pairs


def build_nc():
    nc = bacc.Bacc(None, target_bir_lowering=False)
    dF1 = nc.dram_tensor("F1", [N, C], F32, kind="ExternalInput")
    dF2 = nc.dram_tensor("F2", [M, C], F32, kind="ExternalInput")
    dW = nc.dram_tensor("Wqkv", [C, C], F32, kind="ExternalInput")
    dBq = nc.dram_tensor("bqkv", [1, C], F32, kind="ExternalInput")
    dWp = nc.dram_tensor("Wproj", [C, C], F32, kind="ExternalInput")
    dBp = nc.dram_tensor("bproj", [1, C], F32, kind="ExternalInput")
    dOut = nc.dram_tensor("OUT", [N, C], F32, kind="ExternalOutput")

    d_ident16 = nc.inline_tensor(np.eye(P, dtype=np.float16), name="identity16")
    d_ident32 = nc.inline_tensor(np.eye(P, dtype=np.float32), name="identity32")
    d_ones2 = nc.inline_tensor(np.ones((P, 2, 32), np.float16), name="ones2")

    with tile.TileContext(nc) as tc:
        with (
            tc.tile_pool(name="const", bufs=1) as const,
            tc.tile_pool(name="persist", bufs=1) as persist,
            tc.tile_pool(name="wtmp", bufs=2) as wtmp,
            tc.tile_pool(name="fin", bufs=6) as fpool,
        ):
            # ---- constants (sync queue; F casting DMAs own gpsimd q) ----
            ident16 = const.tile([P, P], F16)
            nc.sync.dma_start(ident16, d_ident16[:])
            ident32 = const.tile([P, P], F32)
            nc.sync.dma_start(ident32, d_ident32[:])
            ones16 = const.tile([P, 2, 32], F16)
            nc.sync.dma_start(ones16, d_ones2[:])
            ones8 = const.tile([P, 2, 32], FP8)
            with nc.allow_low_precision(reason="ones are exact in fp8"):
                nc.vector.tensor_copy(ones8, ones16)
            bq_col = const.tile([P, KC], F32)
            nc.sync.dma_start(bq_col, dBq[0, :].rearrange("(a b) -> b a", b=P))
            bp_row = const.tile([1, C], F32)
            nc.sync.dma_start(bp_row, dBp[:])
            bpb = const.tile([P, C], F32)

            W16, Wp16 = [], []

            def load_w(dsrc, lst, nm):
                for kc in range(KC):
                    w32 = wtmp.tile([P, C], F32, tag="w32", name="w32")
                    nc.sync.dma_start(w32, dsrc[kc * P:(kc + 1) * P, :])
                    w16 = persist.tile([P, C], F16, name=f"{nm}16_{kc}")
                    with nc.allow_low_precision(reason="fp16 weights"):
                        nc.vector.tensor_copy(w16, w32)
                    lst.append(w16)

            load_w(dW, W16, "w")

            # ---- persistent activations ----
            FT = {
                t: [persist.tile([P, N], F16, name=f"{t}T{i}")
                    for i in range(KC)]
                for t in ("f2", "f1")
            }
            # qT8: fp8 d-major q, one zero stripe of pad keeps the
            # (annihilated) second DoubleRow k-tile read in bounds
            qT8 = [persist.tile([P, N + SW], FP8, name=f"qT8_{i}")
                   for i in range(KC)]
            kvT16 = [persist.tile([P, N], F16, name=f"kvT16_{i}")
                     for i in range(KC)]
            # kvT8z: [:,0,:]=kv^T fp8 data, [:,1,:]=0 (kills k-tile 1)
            kvT8z = [persist.tile([P, 2, N], FP8, name=f"kvT8z_{i}")
                     for i in range(KC)]
            kvn2 = persist.tile([P, JJ, 2, C], FP8, name="kvn2")
            xT = [persist.tile([P, N], F16, name=f"xT{i}") for i in range(KC)]
            for h in range(H):
                # zero ALL of qT8, not just the pad: the annihilated second
                # DoubleRow k-tile reads the next stripe before it is
                # written, and uninitialized fp8 bytes can be NaN (0*NaN=NaN)
                nc.vector.memset(qT8[h][:, :], 0.0)
                nc.vector.memset(kvT8z[h][:, 1, :], 0.0)

            with tc.tile_pool(name="pj", bufs=2, space="PSUM") as pjps:

                def proj_chunk(co, g, is_q):
                    src = FT["f1"] if is_q else FT["f2"]
                    pjt = pjps.tile([P, SW], F32, tag="pj", name="pjt")
                    for kc in range(KC):
                        nc.tensor.matmul(
                            pjt,
                            W16[kc][:, co * P:(co + 1) * P],
                            src[kc][:, g * SW:(g + 1) * SW],
                            start=(kc == 0),
                            stop=(kc == KC - 1),
                        )
                    gs = slice(g * SW, (g + 1) * SW)
                    with nc.allow_low_precision(reason="fp8 attention"):
                        if is_q:
                            nc.vector.tensor_scalar_add(
                                qT8[co][:, gs], pjt, bq_col[:, co:co + 1]
                            )
                        else:
                            nc.vector.tensor_scalar_add(
                                kvT16[co][:, gs], pjt, bq_col[:, co:co + 1]
                            )
                            nc.vector.tensor_scalar_add(
                                kvT8z[co][:, 0, gs], pjt,
                                bq_col[:, co:co + 1],
                            )

                def kvn_half(hh, half):
                    # kvn2[p,jj,j,hh*128+d] = kv[(2jj+j)*128+p, hh*128+d]
                    kt = pjps.tile([P, 8, P], F16, tag="pj", name="kt")
                    for u in range(8):
                        mb = 8 * half + u
                        nc.tensor.transpose(
                            kt[:, u, :],
                            kvT16[hh][:, mb * P:(mb + 1) * P],
                            ident16,
                        )
                    for v in range(4):
                        jj = 4 * half + v
                        with nc.allow_low_precision(
                            reason="fp8 kv for DoubleRow pv"
                        ):
                            nc.vector.tensor_copy(
                                kvn2[:, jj, :, hh * P:(hh + 1) * P],
                                kt[:, 2 * v:2 * v + 2, :],
                            )

                def f1t_half(g, half):
                    # F1^T for column group g, kc-pair `half`, via the
                    # shared pj PSUM ring (runs inside attention)
                    tp = pjps.tile([P, 2, SW], F16, tag="pj", name="tp")
                    for i in range(4):
                        nb = 4 * g + i
                        fin = fpool.tile([P, C // 2], F16, tag="finh",
                                         name="finh")
                        nc.gpsimd.dma_start(
                            fin,
                            dF1[nb * P:(nb + 1) * P,
                                half * 2 * P:(half + 1) * 2 * P],
                        )
                        for k in range(2):
                            nc.tensor.transpose(
                                tp[:, k, i * P:(i + 1) * P],
                                fin[:, k * P:(k + 1) * P],
                                ident16,
                            )
                    for k in range(2):
                        kc = 2 * half + k
                        nc.vector.tensor_copy(
                            FT["f1"][kc][:, g * SW:(g + 1) * SW],
                            tp[:, k, :],
                        )

                # ---- phase 1 head ----
                with tc.tile_pool(name="trps", bufs=2, space="PSUM") as trps:
                    def ft16_group(dsrc, tag, g):
                        tp = trps.tile([P, KC, SW], F16, tag="trp", name="tp")
                        for i in range(4):
                            nb = 4 * g + i
                            fin = fpool.tile([P, C], F16, tag="fin",
                                             name="fin")
                            nc.gpsimd.dma_start(
                                fin, dsrc[nb * P:(nb + 1) * P, :]
                            )
                            for kc in range(KC):
                                nc.tensor.transpose(
                                    tp[:, kc, i * P:(i + 1) * P],
                                    fin[:, kc * P:(kc + 1) * P],
                                    ident16,
                                )
                        for kc in range(KC):
                            nc.vector.tensor_copy(
                                FT[tag][kc][:, g * SW:(g + 1) * SW],
                                tp[:, kc, :],
                            )

                    def ft32_group(dsrc, tag, g):
                        # fp32 chunks from the sync queue: transpose at
                        # 2 cyc/row, cast to fp16 during the evac
                        for hf in range(2):
                            tp = trps.tile([P, 2, SW], F32, tag="trp",
                                           name="tp32")
                            for i in range(4):
                                nb = 4 * g + i
                                fin = fpool.tile([P, C // 2], F32,
                                                 tag="fin32", name="fin32")
                                nc.sync.dma_start(
                                    fin,
                                    dsrc[nb * P:(nb + 1) * P,
                                         hf * 2 * P:(hf + 1) * 2 * P],
                                )
                                for k in range(2):
                                    nc.tensor.transpose(
                                        tp[:, k, i * P:(i + 1) * P],
                                        fin[:, k * P:(k + 1) * P],
                                        ident32,
                                    )
                            for k in range(2):
                                kc = 2 * hf + k
                                nc.vector.tensor_copy(
                                    FT[tag][kc][:, g * SW:(g + 1) * SW],
                                    tp[:, k, :],
                                )

                    # F2 (gpsimd casting DMAs) woven with kvT head 0;
                    # F1-g0 fp32 arrives on the sync queue in parallel
                    for g in range(NS):
                        ft16_group(dF2, "f2", g)
                        proj_chunk(0, g, is_q=False)
                    ft32_group(dF1, "f1", 0)
                    load_w(dWp, Wp16, "wp")
                    kvn_half(0, 0)
                    kvn_half(0, 1)
                    proj_chunk(0, 0, is_q=True)

                # bpb broadcast late: keeps the gpsimd queue head free
                # for the F casting DMAs
                nc.gpsimd.partition_broadcast(bpb, bp_row)

                # per-iteration filler: all of it dependency-safe
                # (iter t is (s,h) = divmod(t, H); head co's kvT/kvn are
                # done by end of iter co-1; f1t(g) before qT(:,g) use)
                filler = {
                    0: [("kvT", 1, 0), ("kvT", 1, 1), ("kvT", 1, 2),
                        ("kvT", 1, 3), ("kvn", 1, 0), ("kvn", 1, 1)],
                    1: [("kvT", 2, 0), ("kvT", 2, 1), ("kvT", 2, 2),
                        ("kvT", 2, 3), ("kvn", 2, 0), ("kvn", 2, 1)],
                    2: [("kvT", 3, 0), ("kvT", 3, 1), ("kvT", 3, 2),
                        ("kvT", 3, 3), ("kvn", 3, 0), ("kvn", 3, 1),
                        ("f1t", 1, 0), ("f1t", 1, 1)],
                    5: [("f1t", 2, 0), ("f1t", 2, 1)],
                    9: [("f1t", 3, 0), ("f1t", 3, 1)],
                }
                qT_done = {(0, 0)}
                ph4_pend = []

                # ---- phase 3+4: attention ----
                with (
                    tc.tile_pool(name="scps", bufs=2, space="PSUM") as scps,
                    tc.tile_pool(name="pvps", bufs=1, space="PSUM") as pvps,
                    tc.tile_pool(name="dnps", bufs=1, space="PSUM") as dnps,
                    tc.tile_pool(name="epool", bufs=2) as epool,
                    tc.tile_pool(name="ep", bufs=2) as ep,
                    tc.tile_pool(name="osb", bufs=3) as osb,
                ):
                    def ph4_chunk(nb):
                        pr = pjps.tile([P, C], F32, tag="pj", name="pr")
                        for kc in range(KC):
                            nc.tensor.matmul(
                                pr,
                                xT[kc][:, nb * P:(nb + 1) * P],
                                Wp16[kc],
                                start=(kc == 0),
                                stop=(kc == KC - 1),
                            )
                        ot = osb.tile([P, C], F32, tag="ot", name="ot")
                        nc.vector.tensor_add(ot, pr, bpb)
                        nc.sync.dma_start(dOut[nb * P:(nb + 1) * P, :], ot)

                    def emit_item(it):
                        kind = it[0]
                        if kind == "kvT":
                            proj_chunk(it[1], it[2], is_q=False)
                        elif kind == "kvn":
                            kvn_half(it[1], it[2])
                        elif kind == "f1t":
                            f1t_half(it[1], it[2])
                        elif kind == "qT":
                            proj_chunk(it[1], it[2], is_q=True)
                            qT_done.add((it[1], it[2]))

                    def emit_pv_dn(st, jp):
                        if jp == 0:
                            st["pvp"] = pvps.tile([P, SW], F32, tag="pv",
                                                  name="pvp")
                            st["dnp"] = dnps.tile([32, SW], F32, tag="dn",
                                                  name="dnp")
                        E, h = st["E"], st["h"]
                        nc.tensor.matmul(
                            st["pvp"],
                            kvn2[:, jp, :, h * P:(h + 1) * P],
                            E[:, 2 * jp:2 * jp + 2, :],
                            start=(jp == 0),
                            stop=(jp == JJ - 1),
                            perf_mode=DR,
                        )
                        nc.tensor.matmul(
                            st["dnp"],
                            ones8,
                            E[:, 2 * jp:2 * jp + 2, :],
                            start=(jp == 0),
                            stop=(jp == JJ - 1),
                            perf_mode=DR,
                        )

                    def emit_epilogue(st):
                        h, s = st["h"], st["s"]
                        pvs = ep.tile([P, SW], F16, tag="pvs", name="pvs")
                        with nc.allow_low_precision(reason="x in fp16"):
                            nc.vector.tensor_copy(pvs, st["pvp"])
                        rec = ep.tile([1, SW], F32, tag="rec", name="rec")
                        nc.vector.reciprocal_approx_fast(
                            rec, st["dnp"][0:1, :])
                        dnb = ep.tile([P, SW], F32, tag="dnb", name="dnb")
                        nc.gpsimd.partition_broadcast(dnb, rec)
                        with nc.allow_low_precision(
                            reason="x values O(0.1); fp16 keeps 5e-4 rel"
                        ):
                            nc.vector.tensor_mul(
                                xT[h][:, s * SW:(s + 1) * SW], pvs, dnb
                            )

                    prev = None
                    for t in range(NS * H):
                        s, h = divmod(t, H)
                        if (h, s) not in qT_done:
                            emit_item(("qT", h, s))
                        E = epool.tile([P, MB, SW], FP8, tag="E", name="E")
                        cur = {"E": E, "h": h, "s": s}
                        for jp in range(JJ):
                            sc = scps.tile([P, 2, SW], F32, tag="sc",
                                           name="sc")
                            for i in range(2):
                                mb = 2 * jp + i
                                nc.tensor.matmul(
                                    sc[:, i, :],
                                    kvT8z[h][:, :, mb * P:(mb + 1) * P],
                                    qT8[h][:, s * SW:(s + 2) * SW].rearrange(
                                        "p (a b) -> p a b", a=2
                                    ),
                                    start=True,
                                    stop=True,
                                    perf_mode=DR,
                                )
                            with nc.allow_low_precision(
                                reason="fp8 attention weights; ~1.5e-2 rel"
                            ):
                                nc.scalar.activation(
                                    E[:, 2 * jp:2 * jp + 2, :].rearrange(
                                        "p a b -> p (a b)"
                                    ),
                                    sc.rearrange("p a b -> p (a b)"),
                                    EXP,
                                    scale=float(SCALE),
                                )
                            if prev is not None:
                                emit_pv_dn(prev, jp)
                        if prev is not None:
                            emit_epilogue(prev)
                            if prev["h"] == H - 1:
                                ph4_pend.extend(
                                    range(4 * prev["s"], 4 * prev["s"] + 4))
                        for it in filler.pop(t, ()):
                            emit_item(it)
                        if t + 1 < NS * H:
                            s2, h2 = divmod(t + 1, H)
                            if (h2, s2) not in qT_done:
                                emit_item(("qT", h2, s2))
                        n_ph4 = 2 if ph4_pend else 0
                        for nb in ph4_pend[:n_ph4]:
                            ph4_chunk(nb)
                        ph4_pend = ph4_pend[n_ph4:]
                        prev = cur
                    for jp in range(JJ):
                        emit_pv_dn(prev, jp)
                    emit_epilogue(prev)
                    for nb in ph4_pend:
                        ph4_chunk(nb)
                    ph4_chunk(NB - 4)
                    ph4_chunk(NB - 3)
                    ph4_chunk(NB - 2)
                    ph4_chunk(NB - 1)

    nc.compile()
    return nc


_NC = None


def _get_nc():
    global _NC
    if _NC is None:
        _NC = build_nc()
    return _NC


def kernel(F1, F2, W_qkv, b_qkv, W_proj, b_proj, _trace=False):
    F1 = np.ascontiguousarray(np.asarray(F1, dtype=np.float32))
    F2 = np.ascontiguousarray(np.asarray(F2, dtype=np.float32))
    W = np.ascontiguousarray(np.asarray(W_qkv, dtype=np.float32))
    bq = np.ascontiguousarray(np.asarray(b_qkv, dtype=np.float32)).reshape(1, C)
    Wpj = np.ascontiguousarray(np.asarray(W_proj, dtype=np.float32))
    bp = np.ascontiguousarray(np.asarray(b_proj, dtype=np.float32)).reshape(1, C)

    nc = _get_nc()
    in_maps = [
        {"F1": F1[b], "F2": F2[b], "Wqkv": W, "bqkv": bq, "Wproj": Wpj, "bproj": bp}
        for b in range(B)
    ]
    res = run_bass_kernel_spmd(
        nc, in_maps, core_ids=list(range(B)), trace=_trace
    )
    out = np.stack([res.results[b]["OUT"] for b in range(B)], axis=0)
    if _trace:
        return out, res
    return out


# revision 11
# speedup vs baseline: 1.1096x; 1.1096x over previous
"""Trainium2 Bass kernel for nn_CrossAttention (B=8, N=M=2048, C=512, H=4).

Sharding: data-parallel over batch - one batch element per NeuronCore (8 cores).

v5. Two limits shape the schedule: the scalar engine's exp over all
N*M*H scores (16.8M elems ~ 137us at 128 lanes * 1.2 GHz) and the tensor
engine's ~185us of matmul stream + unhidden per-matmul LDWEIGHTS. Both
engines are kept near-100% busy:

  1. Inputs land via two parallel DMA paths: F2 and F1-g1..3 as
     fp32->fp16 casting DMAs on the gpsimd queue; W / F1-g0 / W_proj as
     fp32 on the sync queue (fp32 PE transposes for F1-g0). The head is
     emitted interleaved - F2^T groups, kvT head-0 chunks, F1-g0^T
     halves, kvT head-1 chunks - so the in-order PE queue always has
     ready work while the F2 DMA streams in.
  2. qT/kvT = (F @ W + b)^T fp16 d-major (bias fused into DVE evac);
     kvn2 = m-major kv mb-pairs [P, jj, 2, C] fp8e4 (DoubleRow
     stationary for pv), from PE transposes of kvT.
  3. Attention per (stripe of 512, head), pipelined one iteration deep:
       scores: fp16 matmuls (fp8 DoubleRow measured slower here: the
               256-cycle matmuls are LDWEIGHTS/issue-bound).
       exp:    ACT, PSUM->SBUF fp8e4 E (|SCALE*s| <= ~2, no max-sub).
       pv/dn:  fp8 DoubleRow over mb-pairs (0.5 cyc/col); dn via a
               [128,2,32] ones stationary into a [32,SW] PSUM block.
     pv/dn of iteration t-1 interleave between the scores of iteration
     t (PSUM: sc 2x2 + pv 1 + dn 1 + pj 2 = 8 banks). The epilogue
     copies pv to SBUF first (frees its PSUM bank without waiting on
     the reciprocal), then reciprocal_approx_fast on [1,SW], GPSIMD
     partition-broadcast, DVE multiply into xT.
     Remaining phase-2 work (kvT heads 2-3, kvn, F1^T g1-3, qT chunks
     just-in-time+1) and phase-4 output chunks drain one item per
     jp-slot / iteration, filling PE slack under ACT's exp cadence.
  4. out[n,c] = xT.T @ W_proj + b (fp16 PE; bias added by DVE from a
     pre-broadcast tile), spread one n-block at a time, DMA out.
"""
import sys

for _p in ("/opt/trn_rl_repo", "/root/.axon_site/_ro/trn_rl_repo"):
    if _p not in sys.path:
        sys.path.insert(0, _p)

import numpy as np
import concourse.bass as bass
import concourse.bacc as bacc
import concourse.tile as tile
from concourse import mybir
from concourse.bass_utils import run_bass_kernel_spmd

F32 = mybir.dt.float32
F16 = mybir.dt.float16
FP8 = mybir.dt.float8e4
EXP = mybir.ActivationFunctionType.Exp
DR = mybir.MatmulPerfMode.DoubleRow

B, N, M, C = 8, 2048, 2048, 512
H, D = 4, 128
SCALE = 1.0 / np.sqrt(C)
P = 128
NB = N // P        # 16 n-blocks
MB = M // P        # 16 m-blocks
KC = C // P        # 4 contraction chunks (= heads since D=128)
NS = 4             # n-stripes of 512
SW = N // NS       # stripe width 512
JJ = MB // 2       # 8 mb-pairs


def build_nc():
    nc = bacc.Bacc(None, target_bir_lowering=False)
    dF1 = nc.dram_tensor("F1", [N, C], F32, kind="ExternalInput")
    dF2 = nc.dram_tensor("F2", [M, C], F32, kind="ExternalInput")
    dW = nc.dram_tensor("Wqkv", [C, C], F32, kind="ExternalInput")
    dBq = nc.dram_tensor("bqkv", [1, C], F32, kind="ExternalInput")
    dWp = nc.dram_tensor("Wproj", [C, C], F32, kind="ExternalInput")
    dBp = nc.dram_tensor("bproj", [1, C], F32, kind="ExternalInput")
    dOut = nc.dram_tensor("OUT", [N, C], F32, kind="ExternalOutput")

    d_ident16 = nc.inline_tensor(np.eye(P, dtype=np.float16), name="identity16")
    d_ident32 = nc.inline_tensor(np.eye(P, dtype=np.float32), name="identity32")
    d_ones2 = nc.inline_tensor(np.ones((P, 2, 32), np.float16), name="ones2")

    with tile.TileContext(nc) as tc:
        with (
            tc.tile_pool(name="const", bufs=1) as const,
            tc.tile_pool(name="persist", bufs=1) as persist,
            tc.tile_pool(name="wtmp", bufs=2) as wtmp,
            tc.tile_pool(name="fin", bufs=6) as fpool,
        ):
            # ---- constants (sync queue; F casting DMAs own gpsimd q) ----
            ident16 = const.tile([P, P], F16)
            nc.sync.dma_start(ident16, d_ident16[:])
            ident32 = const.tile([P, P], F32)
            nc.sync.dma_start(ident32, d_ident32[:])
            ones16 = const.tile([P, 2, 32], F16)
            nc.sync.dma_start(ones16, d_ones2[:])
            ones8 = const.tile([P, 2, 32], FP8)
            with nc.allow_low_precision(reason="ones are exact in fp8"):
                nc.vector.tensor_copy(ones8, ones16)
            bq_col = const.tile([P, KC], F32)
            nc.sync.dma_start(bq_col, dBq[0, :].rearrange("(a b) -> b a", b=P))
            bp_row = const.tile([1, C], F32)
            nc.sync.dma_start(bp_row, dBp[:])
            bpb = const.tile([P, C], F32)

            W16, Wp16 = [], []

            def load_w(dsrc, lst, nm):
                for kc in range(KC):
                    w32 = wtmp.tile([P, C], F32, tag="w32", name="w32")
                    nc.sync.dma_start(w32, dsrc[kc * P:(kc + 1) * P, :])
                    w16 = persist.tile([P, C], F16, name=f"{nm}16_{kc}")
                    with nc.allow_low_precision(reason="fp16 weights"):
                        nc.vector.tensor_copy(w16, w32)
                    lst.append(w16)

            load_w(dW, W16, "w")

            # ---- persistent activations ----
            FT = {
                t: [persist.tile([P, N], F16, name=f"{t}T{i}")
                    for i in range(KC)]
                for t in ("f2", "f1")
            }
            qT = [persist.tile([P, N], F16, name=f"qT{i}") for i in range(KC)]
            kvT = [persist.tile([P, N], F16, name=f"kvT{i}") for i in range(KC)]
            kvn2 = persist.tile([P, JJ, 2, C], FP8, name="kvn2")
            xT = [persist.tile([P, N], F16, name=f"xT{i}") for i in range(KC)]

            with tc.tile_pool(name="pj", bufs=2, space="PSUM") as pjps:

                def proj_chunk(co, g, is_q):
                    src = FT["f1"] if is_q else FT["f2"]
                    dst = qT if is_q else kvT
                    pjt = pjps.tile([P, SW], F32, tag="pj", name="pjt")
                    for kc in range(KC):
                        nc.tensor.matmul(
                            pjt,
                            W16[kc][:, co * P:(co + 1) * P],
                            src[kc][:, g * SW:(g + 1) * SW],
                            start=(kc == 0),
                            stop=(kc == KC - 1),
                        )
                    nc.vector.tensor_scalar_add(
                        dst[co][:, g * SW:(g + 1) * SW],
                        pjt,
                        bq_col[:, co:co + 1],
                    )

                def kvn_half(hh, half):
                    # kvn2[p,jj,j,hh*128+d] = kv[(2jj+j)*128+p, hh*128+d]
                    kt = pjps.tile([P, 8, P], F16, tag="pj", name="kt")
                    for u in range(8):
                        mb = 8 * half + u
                        nc.tensor.transpose(
                            kt[:, u, :],
                            kvT[hh][:, mb * P:(mb + 1) * P],
                            ident16,
                        )
                    for v in range(4):
                        jj = 4 * half + v
                        with nc.allow_low_precision(
                            reason="fp8 kv for DoubleRow pv"
                        ):
                            nc.vector.tensor_copy(
                                kvn2[:, jj, :, hh * P:(hh + 1) * P],
                                kt[:, 2 * v:2 * v + 2, :],
                            )

                def f1t_half(g, half):
                    # F1^T column group g, kc-pair `half`, via the shared
                    # pj PSUM ring (gpsimd casting DMA; runs in attention)
                    tp = pjps.tile([P, 2, SW], F16, tag="pj", name="tp")
                    for i in range(4):
                        nb = 4 * g + i
                        fin = fpool.tile([P, C // 2], F16, tag="finh",
                                         name="finh")
                        nc.gpsimd.dma_start(
                            fin,
                            dF1[nb * P:(nb + 1) * P,
                                half * 2 * P:(half + 1) * 2 * P],
                        )
                        for k in range(2):
                            nc.tensor.transpose(
                                tp[:, k, i * P:(i + 1) * P],
                                fin[:, k * P:(k + 1) * P],
                                ident16,
                            )
                    for k in range(2):
                        kc = 2 * half + k
                        nc.vector.tensor_copy(
                            FT["f1"][kc][:, g * SW:(g + 1) * SW],
                            tp[:, k, :],
                        )

                # ---- phase 1 head, interleaved for the in-order PE ----
                with tc.tile_pool(name="trps", bufs=2, space="PSUM") as trps:
                    def ft16_group(dsrc, tag, g):
                        tp = trps.tile([P, KC, SW], F16, tag="trp", name="tp")
                        for i in range(4):
                            nb = 4 * g + i
                            fin = fpool.tile([P, C], F16, tag="fin",
                                             name="fin")
                            nc.gpsimd.dma_start(
                                fin, dsrc[nb * P:(nb + 1) * P, :]
                            )
                            for kc in range(KC):
                                nc.tensor.transpose(
                                    tp[:, kc, i * P:(i + 1) * P],
                                    fin[:, kc * P:(kc + 1) * P],
                                    ident16,
                                )
                        for kc in range(KC):
                            nc.vector.tensor_copy(
                                FT[tag][kc][:, g * SW:(g + 1) * SW],
                                tp[:, kc, :],
                            )

                    def ft32_half(dsrc, tag, g, hf):
                        # fp32 chunks from the sync queue: transpose at
                        # 2 cyc/row, cast to fp16 during the evac
                        tp = trps.tile([P, 2, SW], F32, tag="trp",
                                       name="tp32")
                        for i in range(4):
                            nb = 4 * g + i
                            fin = fpool.tile([P, C // 2], F32,
                                             tag="fin32", name="fin32")
                            nc.sync.dma_start(
                                fin,
                                dsrc[nb * P:(nb + 1) * P,
                                     hf * 2 * P:(hf + 1) * 2 * P],
                            )
                            for k in range(2):
                                nc.tensor.transpose(
                                    tp[:, k, i * P:(i + 1) * P],
                                    fin[:, k * P:(k + 1) * P],
                                    ident32,
                                )
                        for k in range(2):
                            kc = 2 * hf + k
                            nc.vector.tensor_copy(
                                FT[tag][kc][:, g * SW:(g + 1) * SW],
                                tp[:, k, :],
                            )

                    ft16_group(dF2, "f2", 0)
                    proj_chunk(0, 0, is_q=False)
                    ft16_group(dF2, "f2", 1)
                    proj_chunk(0, 1, is_q=False)
                    ft32_half(dF1, "f1", 0, 0)
                    ft16_group(dF2, "f2", 2)
                    proj_chunk(0, 2, is_q=False)
                    ft32_half(dF1, "f1", 0, 1)
                    proj_chunk(1, 0, is_q=False)
                    ft16_group(dF2, "f2", 3)
                    proj_chunk(0, 3, is_q=False)
                    proj_chunk(1, 1, is_q=False)
                    kvn_half(0, 0)
                    kvn_half(0, 1)
                    proj_chunk(0, 0, is_q=True)
                    load_w(dWp, Wp16, "wp")

                # bpb broadcast late: keeps the gpsimd queue head free
                # for the F casting DMAs
                nc.gpsimd.partition_broadcast(bpb, bp_row)

                # per-iteration filler, emitted one item per jp slot
                # (iter t=(s,h): head co's kvT/kvn done by end of iter
                # co-2 or earlier; f1t(g) before qT(:,g) prefetch)
                filler = {
                    0: [("kvT", 1, 2), ("kvT", 1, 3), ("kvn", 1, 0),
                        ("kvn", 1, 1), ("kvT", 2, 0), ("kvT", 2, 1)],
                    1: [("kvT", 2, 2), ("kvT", 2, 3), ("kvn", 2, 0),
                        ("kvn", 2, 1), ("kvT", 3, 0), ("kvT", 3, 1)],
                    2: [("kvT", 3, 2), ("kvT", 3, 3), ("kvn", 3, 0),
                        ("kvn", 3, 1), ("f1t", 1, 0), ("f1t", 1, 1)],
                    5: [("f1t", 2, 0), ("f1t", 2, 1)],
                    9: [("f1t", 3, 0), ("f1t", 3, 1)],
                }
                qT_done = {(0, 0)}
                ph4_pend = []

                # ---- phase 3+4: attention ----
                with (
                    tc.tile_pool(name="scps", bufs=2, space="PSUM") as scps,
                    tc.tile_pool(name="pvps", bufs=1, space="PSUM") as pvps,
                    tc.tile_pool(name="dnps", bufs=1, space="PSUM") as dnps,
                    tc.tile_pool(name="epool", bufs=2) as epool,
                    tc.tile_pool(name="ep", bufs=2) as ep,
                    tc.tile_pool(name="osb", bufs=3) as osb,
                ):
                    def ph4_chunk(nb):
                        pr = pjps.tile([P, C], F32, tag="pj", name="pr")
                        for kc in range(KC):
                            nc.tensor.matmul(
                                pr,
                                xT[kc][:, nb * P:(nb + 1) * P],
                                Wp16[kc],
                                start=(kc == 0),
                                stop=(kc == KC - 1),
                            )
                        ot = osb.tile([P, C], F32, tag="ot", name="ot")
                        nc.vector.tensor_add(ot, pr, bpb)
                        nc.sync.dma_start(dOut[nb * P:(nb + 1) * P, :], ot)

                    def emit_item(it):
                        kind = it[0]
                        if kind == "kvT":
                            proj_chunk(it[1], it[2], is_q=False)
                        elif kind == "kvn":
                            kvn_half(it[1], it[2])
                        elif kind == "f1t":
                            f1t_half(it[1], it[2])
                        elif kind == "qT":
                            proj_chunk(it[1], it[2], is_q=True)
                            qT_done.add((it[1], it[2]))
                        else:
                            ph4_chunk(it[1])

                    def emit_pv_dn(st, jp):
                        if jp == 0:
                            st["pvp"] = pvps.tile([P, SW], F32, tag="pv",
                                                  name="pvp")
                            st["dnp"] = dnps.tile([32, SW], F32, tag="dn",
                                                  name="dnp")
                        E, h = st["E"], st["h"]
                        nc.tensor.matmul(
                            st["pvp"],
                            kvn2[:, jp, :, h * P:(h + 1) * P],
                            E[:, 2 * jp:2 * jp + 2, :],
                            start=(jp == 0),
                            stop=(jp == JJ - 1),
                            perf_mode=DR,
                        )
                        nc.tensor.matmul(
                            st["dnp"],
                            ones8,
                            E[:, 2 * jp:2 * jp + 2, :],
                            start=(jp == 0),
                            stop=(jp == JJ - 1),
                            perf_mode=DR,
                        )

                    def emit_epilogue(st):
                        h, s = st["h"], st["s"]
                        pvs = ep.tile([P, SW], F16, tag="pvs", name="pvs")
                        with nc.allow_low_precision(reason="x in fp16"):
                            nc.vector.tensor_copy(pvs, st["pvp"])
                        rec = ep.tile([1, SW], F32, tag="rec", name="rec")
                        nc.vector.reciprocal_approx_fast(
                            rec, st["dnp"][0:1, :])
                        dnb = ep.tile([P, SW], F32, tag="dnb", name="dnb")
                        nc.gpsimd.partition_broadcast(dnb, rec)
                        with nc.allow_low_precision(
                            reason="x values O(0.1); fp16 keeps 5e-4 rel"
                        ):
                            nc.vector.tensor_mul(
                                xT[h][:, s * SW:(s + 1) * SW], pvs, dnb
                            )

                    prev = None
                    for t in range(NS * H):
                        s, h = divmod(t, H)
                        if (h, s) not in qT_done:
                            emit_item(("qT", h, s))
                        todo = list(filler.pop(t, ()))
                        if t + 1 < NS * H:
                            s2, h2 = divmod(t + 1, H)
                            if (h2, s2) not in qT_done:
                                todo.append(("qT", h2, s2))
                                qT_done.add((h2, s2))
                        n_ph4 = min(2, len(ph4_pend))
                        todo.extend(("ph4", nb) for nb in ph4_pend[:n_ph4])
                        ph4_pend = ph4_pend[n_ph4:]

                        E = epool.tile([P, MB, SW], FP8, tag="E", name="E")
                        cur = {"E": E, "h": h, "s": s}
                        for jp in range(JJ):
                            sc = scps.tile([P, 2, SW], F32, tag="sc",
                                           name="sc")
                            for i in range(2):
                                mb = 2 * jp + i
                                nc.tensor.matmul(
                                    sc[:, i, :],
                                    kvT[h][:, mb * P:(mb + 1) * P],
                                    qT[h][:, s * SW:(s + 1) * SW],
                                    start=True,
                                    stop=True,
                                )
                            with nc.allow_low_precision(
                                reason="fp8 attention weights"
                            ):
                                nc.scalar.activation(
                                    E[:, 2 * jp:2 * jp + 2, :].rearrange(
                                        "p a b -> p (a b)"
                                    ),
                                    sc.rearrange("p a b -> p (a b)"),
                                    EXP,
                                    scale=float(SCALE),
                                )
                            if prev is not None:
                                emit_pv_dn(prev, jp)
                            if todo:
                                emit_item(todo.pop(0))
                        for it in todo:
                            emit_item(it)
                        if prev is not None:
                            emit_epilogue(prev)
                            if prev["h"] == H - 1:
                                ph4_pend.extend(
                                    range(4 * prev["s"], 4 * prev["s"] + 4))
                        prev = cur
                    for jp in range(JJ):
                        emit_pv_dn(prev, jp)
                    emit_epilogue(prev)
                    for nb in ph4_pend:
                        ph4_chunk(nb)
                    for nb in range(NB - 4, NB):
                        ph4_chunk(nb)

    nc.compile()
    return nc


_NC = None


def _get_nc():
    global _NC
    if _NC is None:
        _NC = build_nc()
    return _NC


def kernel(F1, F2, W_qkv, b_qkv, W_proj, b_proj, _trace=False):
    F1 = np.ascontiguousarray(np.asarray(F1, dtype=np.float32))
    F2 = np.ascontiguousarray(np.asarray(F2, dtype=np.float32))
    W = np.ascontiguousarray(np.asarray(W_qkv, dtype=np.float32))
    bq = np.ascontiguousarray(np.asarray(b_qkv, dtype=np.float32)).reshape(1, C)
    Wpj = np.ascontiguousarray(np.asarray(W_proj, dtype=np.float32))
    bp = np.ascontiguousarray(np.asarray(b_proj, dtype=np.float32)).reshape(1, C)

    nc = _get_nc()
    in_maps = [
        {"F1": F1[b], "F2": F2[b], "Wqkv": W, "bqkv": bq, "Wproj": Wpj, "bproj": bp}
        for b in range(B)
    ]
    res = run_bass_kernel_spmd(
        nc, in_maps, core_ids=list(range(B)), trace=_trace
    )
    out = np.stack([res.results[b]["OUT"] for b in range(B)], axis=0)
    if _trace:
        return out, res
    return out
